# revision 37
# baseline (speedup 1.0000x reference)
"""Trainium2 Bass kernel for nn_Baseline_SelfGCN (gnn_message_passing).

Data-parallel over batch: 8 NeuronCores x 8 images each.

v3: DMA-roofline schedule. Big loads stream back-to-back on the Pool
(SWDGE) queue as half-image chunks in priority order: x_gcn (segment
pooling chases it), W1 (layer-1 matmuls chase), x_global (GAP reduces
chase) interleaved with W2 (layer-2 chases), so the only post-stream
tail is the layer-2 epilogue. Mask/param loads ride the SP queue early.
BN folds run in cheap layouts off the critical path and are broadcast
to (b,p)-rows via PE selector matmuls. Segment pooling writes the
layer-1 lhsT directly (PE transpose + one-hot matmul per 128-ch chunk).

Self-contained: hardcodes shapes; host side only shards/gathers.
"""

import numpy as np

import concourse.bass as bass
import concourse.mybir as mybir
import concourse.tile as tile
from concourse.masks import make_identity

F32 = mybir.dt.float32
F32R = mybir.dt.float32r
BF16 = mybir.dt.bfloat16
I32 = mybir.dt.int32
AF = mybir.ActivationFunctionType
OP = mybir.AluOpType

BL = 8          # images per core
C = 2048
HW = 256        # Hf*Wf
NP = 9          # graph nodes (parts 1..9)
R = BL * NP     # 72 rows = (image, part)
EPS = 1e-5
NCH = 4         # 2048 / 512 N-chunks
KT = 16         # 2048 / 128 K-tiles
JH = 8          # j-chunks per half-image load
OUTW = 3 * C + 2 * NP * C  # 43008


def legalize_waits(nc, max_waits=1):
    """Split multi-wait instructions: this walrus build allows only one
    embedded sync-wait per instruction; hoist extras onto standalone
    InstEventSemaphore waits on the same engine."""
    cnt = 0
    for fn in nc.m.functions:
        for blk in fn.blocks:
            out = []
            for inst in blk.instructions:
                si = inst.sync_info
                if si is not None and si.on_wait and len(si.on_wait) > max_waits:
                    waits = list(si.on_wait)
                    for w in waits[:-max_waits]:
                        cnt += 1
                        wi = mybir.InstEventSemaphore(
                            name=f"wsplit{cnt}_{inst.name}", ins=[], outs=[],
                            sync_info=mybir.SyncInfo(on_wait=[w], on_update=[]))
                        wi.engine = inst.engine
                        nc.register_instruction(wi)
                        out.append(wi)
                    si.on_wait = waits[-max_waits:]
                    inst.sync_info = si
                out.append(inst)
            blk.instructions = out
    return cnt


def build_bass():
    nc = bass.Bass()

    xg_p = nc.declare_dram_parameter("x_global", [BL, C, HW], F32, isOutput=False)
    xc_p = nc.declare_dram_parameter("x_gcn", [BL, C, HW], F32, isOutput=False)
    mk_p = nc.declare_dram_parameter("mask", [BL, 256, 256], I32, isOutput=False)
    adj_p = nc.declare_dram_parameter("adj", [BL, NP, NP], F32, isOutput=False)
    w1_p = nc.declare_dram_parameter("W1", [C, C], F32, isOutput=False)
    w2_p = nc.declare_dram_parameter("W2", [C, C], F32, isOutput=False)
    b1_p = nc.declare_dram_parameter("b1", [C], F32, isOutput=False)
    b2_p = nc.declare_dram_parameter("b2", [C], F32, isOutput=False)
    bn1 = {k: nc.declare_dram_parameter(k + "1", [NP * C], F32, isOutput=False)
           for k in ("g", "be", "rm", "rv")}
    bn2 = {k: nc.declare_dram_parameter(k + "2", [NP * C], F32, isOutput=False)
           for k in ("g", "be", "rm", "rv")}
    gb = {k: nc.declare_dram_parameter("gb_" + k, [C], F32, isOutput=False)
          for k in ("g", "b", "rm", "rv")}
    gn = {k: nc.declare_dram_parameter("gn_" + k, [C], F32, isOutput=False)
          for k in ("g", "b", "rm", "rv")}
    out_p = nc.declare_dram_parameter("out", [BL, OUTW], F32, isOutput=True)

    with tile.TileContext(nc) as tc:
        with (
            tc.tile_pool(name="consts", bufs=1) as cs,
            tc.tile_pool(name="ps", bufs=8, space="PSUM") as ps,
            tc.tile_pool(name="stream", bufs=5) as stream,
            tc.tile_pool(name="xtp", bufs=2) as xtp,
            tc.tile_pool(name="wp", bufs=4) as wp,
            tc.tile_pool(name="mm", bufs=1) as mm,
            tc.tile_pool(name="small", bufs=2) as sp,
            tc.tile_pool(name="dram", bufs=1, space="DRAM") as dp,
        ):
            # ================= constants (no DMA deps) =================
            ident = cs.tile([128, 128], F32)
            make_identity(nc, ident[:])
            identr = cs.tile([128, 128], F32R)
            nc.vector.tensor_copy(out=identr[:], in_=ident[:])

            iota_i = cs.tile([128, NP], I32)
            nc.gpsimd.iota(iota_i[:], pattern=[[1, NP]], base=1, channel_multiplier=0)
            iota_f = cs.tile([128, NP], F32)
            nc.vector.tensor_copy(out=iota_f[:], in_=iota_i[:])

            ones_col = cs.tile([128, 1], F32)
            nc.vector.memset(ones_col[:], 1.0)
            ones_row8 = cs.tile([1, BL], BF16)
            nc.vector.memset(ones_row8[:], 1.0)

            # strictly-lower-triangular L9: L[q,p] = 1 if q < p
            L9 = cs.tile([NP, NP], F32)
            nc.gpsimd.memset(L9[:], 0.0)
            nc.gpsimd.affine_select(
                out=L9[:], in_=L9[:], compare_op=OP.is_ge, fill=1.0,
                base=0, pattern=[[-1, NP]], channel_multiplier=1,
            )

            # block "mean over parts" matrix (72, 8): 1/9 on image blocks
            onesblk = cs.tile([R, BL], F32)
            nc.gpsimd.memset(onesblk[:], 1.0 / NP)
            nc.gpsimd.affine_select(
                out=onesblk[:], in_=onesblk[:], compare_op=OP.is_ge, fill=0.0,
                base=0, pattern=[[-NP, BL]], channel_multiplier=1)
            nc.gpsimd.affine_select(
                out=onesblk[:], in_=onesblk[:], compare_op=OP.is_ge, fill=0.0,
                base=NP - 1, pattern=[[NP, BL]], channel_multiplier=-1)

            # selector sel[q, (b,p)] = 1 iff p == q  (for BN-rep broadcast)
            self_f = cs.tile([NP, BL, NP], F32)
            nc.gpsimd.memset(self_f[:], 1.0)
            nc.gpsimd.affine_select(
                out=self_f[:], in_=self_f[:], compare_op=OP.is_ge, fill=0.0,
                base=0, pattern=[[0, BL], [1, NP]], channel_multiplier=-1)
            nc.gpsimd.affine_select(
                out=self_f[:], in_=self_f[:], compare_op=OP.is_ge, fill=0.0,
                base=0, pattern=[[0, BL], [-1, NP]], channel_multiplier=1)
            selb = cs.tile([NP, R], BF16)
            nc.vector.tensor_copy(out=selb[:],
                                  in_=self_f[:].rearrange("q b p -> q (b p)"))
            selr = cs.tile([NP, R], F32R)
            nc.vector.tensor_copy(out=selr[:],
                                  in_=self_f[:].rearrange("q b p -> q (b p)"))
            onesblk_r = cs.tile([R, BL], F32R)
            nc.vector.tensor_copy(out=onesblk_r[:], in_=onesblk[:])

            mrec72 = cs.tile([R, 1], F32)
            selfm72 = cs.tile([R, 1], F32)
            rec98 = cs.tile([NP, BL], F32)
            selfm98 = cs.tile([NP, BL], F32)
            scr_sm = dp.tile([NP, BL], F32, name="scr_sm")
            scr_rc = dp.tile([NP, BL], F32, name="scr_rc")
            G = cs.tile([128, BL, KT], F32)
            Gn = cs.tile([128, BL, KT], F32)
            GT = cs.tile([128, 128], F32)
            dump = cs.tile([128, HW], BF16)

            BD = cs.tile([R, R], F32)
            BDr = cs.tile([R, R], F32R)
            BDs = cs.tile([R, R], F32R)

            srep1 = cs.tile([R, C], BF16)
            srep2 = cs.tile([R, C], BF16)
            BDre = cs.tile([R + NP, R], F32R)   # [BDr; sel] bias rows
            BDse = cs.tile([R + NP, R], F32R)   # [BDs; sel]
            sgb = cs.tile([128, KT], F32)
            tgb = cs.tile([128, KT], F32)
            sgn = cs.tile([BL, C], BF16)
            tgn = cs.tile([BL, C], BF16)

            # =============== early SP-queue DMAs: masks then bn params ======
            mrs = []
            for b in range(BL):
                msrc = mk_p[b, ::16, ::16]  # (16,16)
                mr = sp.tile([128, 2], I32, tag="mr", name=f"mr{b}", bufs=8)
                nc.sync.dma_start(out=mr[:, 0:1], in_=msrc[0:8, :])
                nc.sync.dma_start(out=mr[:, 1:2], in_=msrc[8:16, :])
                mrs.append(mr)

            bnt = {}
            for li, (bnp, blin) in enumerate(((bn1, b1_p), (bn2, b2_p))):
                for k in ("g", "be", "rm", "rv"):
                    t = sp.tile([72, 256], F32, tag=f"bn_{k}", name=f"bn{li}_{k}",
                                bufs=1)
                    nc.sync.dma_start(
                        out=t[:], in_=bnp[k].rearrange("(k lo) -> k lo", lo=256))
                    bnt[(li, k)] = t
                t = sp.tile([72, 256], F32, tag="bn_b", name=f"bn{li}_b", bufs=1)
                bv = blin.rearrange("(j lo) -> j lo", lo=256)[:]
                nc.sync.dma_start(
                    out=t[:],
                    in_=bass.AP(tensor=bv.tensor, offset=bv.offset,
                                ap=[[0, NP]] + list(bv.ap)))
                bnt[(li, "b")] = t

            # ================= mask chain helper (emitted per image) ========
            ohs = {}

            def mask_chain(b):
                mrf = sp.tile([128, 2], F32, tag="mrf", name=f"mrf{b}", bufs=8)
                nc.vector.tensor_copy(out=mrf[:], in_=mrs[b][:])
                oh = []
                for h in range(2):
                    oht = sp.tile([128, NP], F32R, tag=f"oh{h}",
                                  name=f"oh{b}_{h}", bufs=8)
                    nc.vector.tensor_scalar(
                        out=oht[:], in0=iota_f[:], scalar1=mrf[:, h:h + 1],
                        scalar2=None, op0=OP.is_equal)
                    oh.append(oht)
                ohs[b] = oh

                pcnt = ps.tile([NP, 1], F32, tag="ps", name=f"pcnt{b}")
                for h in range(2):
                    nc.tensor.matmul(pcnt[:], oh[h][:].bitcast(F32), ones_col[:],
                                     start=(h == 0), stop=(h == 1))
                nc.vector.tensor_scalar_add(rec98[:, b:b + 1], pcnt[:], 1e-8)
                nc.vector.reciprocal(out=rec98[:, b:b + 1], in_=rec98[:, b:b + 1])
                pres = sp.tile([NP, 1], F32, tag="pres", name=f"pres{b}", bufs=8)
                nc.vector.tensor_scalar(out=pres[:], in0=pcnt[:], scalar1=0.0,
                                        scalar2=None, op0=OP.is_gt)
                ppre = ps.tile([NP, 1], F32, tag="ps", name=f"ppre{b}")
                nc.tensor.matmul(ppre[:], L9[:], pres[:], start=True, stop=True)
                isz = sp.tile([NP, 1], F32, tag="isz", name=f"isz{b}", bufs=8)
                nc.vector.tensor_scalar(out=isz[:], in0=ppre[:], scalar1=0.0,
                                        scalar2=None, op0=OP.is_equal)
                nc.vector.tensor_mul(isz[:], isz[:], pres[:])
                nc.vector.tensor_scalar(out=selfm98[:, b:b + 1], in0=isz[:],
                                        scalar1=-1.0, scalar2=1.0,
                                        op0=OP.mult, op1=OP.add)

            # ============ x_gcn stream (half images) + pooling -> mfT =======
            mfT = mm.tile([128, KT, R], F32R, tag="mfT")
            cp = [0]
            # 9 DVE : 7 Act per 16 copies (Act copies cost ~13% more)
            _dve_slots = {0, 2, 4, 6, 8, 10, 12, 14, 15}

            def copy_ps(dst, srcp):
                if cp[0] % 16 in _dve_slots:
                    nc.vector.tensor_copy(out=dst, in_=srcp)
                else:
                    nc.scalar.copy(out=dst, in_=srcp)
                cp[0] += 1

            for b in range(BL):
                mask_chain(b)
                oh = ohs[b]
                for hf in range(2):
                    xct = stream.tile([128, JH, HW], F32R, tag="stream",
                                      name=f"xc{b}_{hf}", bufs=5)
                    nc.gpsimd.dma_start(
                        out=xct[:],
                        in_=xc_p[b, 128 * JH * hf:128 * JH * (hf + 1), :]
                        .rearrange("(j p) hw -> p j hw", p=128))
                    for g in range(JH // 4):
                        xts = []
                        for dj in range(4):
                            j = 4 * g + dj
                            pair = []
                            for h in range(2):
                                pst = ps.tile([128, 128], F32R, tag="ps",
                                              name=f"pst{b}_{hf}_{j}_{h}")
                                nc.tensor.transpose(
                                    pst[:], xct[:, j, 128 * h:128 * (h + 1)],
                                    identr[:])
                                xt = xtp.tile([128, 128], F32R,
                                              tag=f"xt{dj}_{h}",
                                              name=f"xt{b}_{hf}_{j}_{h}", bufs=2)
                                copy_ps(xt[:], pst[:])
                                pair.append(xt)
                            xts.append(pair)
                        for dj in range(4):
                            j = 4 * g + dj
                            psg = ps.tile([128, NP], F32, tag="ps",
                                          name=f"psg{b}_{hf}_{j}")
                            for h in range(2):
                                nc.tensor.matmul(psg[:],
                                                 xts[dj][h][:].bitcast(F32),
                                                 oh[h][:].bitcast(F32),
                                                 start=(h == 0), stop=(h == 1))
                            copy_ps(mfT[:, JH * hf + j, NP * b:NP * (b + 1)],
                                    psg[:])

            # ---------------- BN folds (cheap layouts, off critical path) ---
            st9, tt9 = {}, {}
            for li in (0, 1):
                sf = sp.tile([72, 256], F32, tag="f_s", name=f"sf{li}", bufs=1)
                tf = sp.tile([72, 256], F32, tag="f_t", name=f"tf{li}", bufs=1)
                nc.vector.tensor_scalar_add(sf[:], bnt[(li, "rv")][:], EPS)
                nc.scalar.activation(out=sf[:], in_=sf[:], func=AF.Sqrt)
                nc.vector.reciprocal(out=sf[:], in_=sf[:])
                nc.vector.tensor_mul(sf[:], sf[:], bnt[(li, "g")][:])
                nc.vector.tensor_sub(tf[:], bnt[(li, "b")][:], bnt[(li, "rm")][:])
                nc.vector.tensor_mul(tf[:], tf[:], sf[:])
                nc.vector.tensor_add(tf[:], tf[:], bnt[(li, "be")][:])
                sfb = sp.tile([72, 256], BF16, tag="f_sb", name=f"sfb{li}", bufs=1)
                nc.vector.tensor_copy(out=sfb[:], in_=sf[:])
                # t/s in f32r: bias rows appended to the bmm contraction
                tqf = sp.tile([72, 256], F32, tag="f_tqf", name=f"tqf{li}", bufs=1)
                nc.vector.reciprocal(out=tqf[:], in_=sf[:])
                nc.vector.tensor_mul(tqf[:], tqf[:], tf[:])
                tq = sp.tile([72, 256], F32R, tag="f_tq", name=f"tq{li}", bufs=2)
                nc.vector.tensor_copy(out=tq[:], in_=tqf[:])
                s9 = cs.tile([NP, C], BF16, name=f"st9_{li}")
                nc.sync.dma_start(out=s9[:], in_=sfb[:])
                st9[li], tt9[li] = s9, tq

            # row scales: transpose (9,8) -> (72,1) via DRAM bounce
            nc.sync.dma_start(out=scr_sm[:], in_=selfm98[:])
            nc.sync.dma_start(out=scr_rc[:], in_=rec98[:])
            smv, rcv = scr_sm[:], scr_rc[:]
            nc.sync.dma_start(
                out=mrec72[:],
                in_=bass.AP(tensor=rcv.tensor, offset=rcv.offset,
                            ap=[[1, BL], [BL, NP], [0, 1]]))
            nc.sync.dma_start(
                out=selfm72[:],
                in_=bass.AP(tensor=smv.tensor, offset=smv.offset,
                            ap=[[1, BL], [BL, NP], [0, 1]]))

            # block-diag adj^T via direct block DMAs
            nc.gpsimd.memset(BD[:], 0.0)
            for b in range(BL):
                nc.sync.dma_start(
                    out=BD[NP * b:NP * (b + 1), NP * b:NP * (b + 1)],
                    in_=adj_p[b].transpose([1, 0]))
            nc.vector.tensor_copy(out=BDr[:], in_=BD[:])
            nc.vector.tensor_scalar(out=BDs[:], in0=BDr[:],
                                    scalar1=selfm72[:, 0:1], scalar2=None,
                                    op0=OP.mult)
            nc.sync.dma_start(out=BDre[0:R, :], in_=BDr[:])
            nc.sync.dma_start(out=BDre[R:R + NP, :], in_=selr[:])
            nc.sync.dma_start(out=BDse[0:R, :], in_=BDs[:])
            nc.sync.dma_start(out=BDse[R:R + NP, :], in_=selr[:])

            # ================= GCN layer 1 (chases W1 stream) =================
            psl1 = [ps.tile([R, 512], F32, tag="ps", name=f"psl1_{n}")
                    for n in range(NCH)]
            for kt in range(KT):
                w = wp.tile([128, C], F32R, tag="w", name=f"w1_{kt}")
                nc.gpsimd.dma_start(out=w[:], in_=w1_p[128 * kt:128 * (kt + 1), :])
                for n in range(NCH):
                    nc.tensor.matmul(psl1[n][:], mfT[:, kt, :],
                                     w[:, 512 * n:512 * (n + 1)],
                                     start=(kt == 0), stop=(kt == KT - 1))

            # ====== BN-scale broadcasts: srep = sel^T @ fold rows ======
            for li, dst in ((0, srep1), (1, srep2)):
                for n in range(NCH):
                    sl = slice(512 * n, 512 * (n + 1))
                    pr = ps.tile([R, 512], F32, tag="ps",
                                 name=f"prep{li}_{dst.name}_{n}")
                    nc.tensor.matmul(pr[:], selb[:], st9[li][:, sl],
                                     start=True, stop=True)
                    if n % 2 == 0:
                        nc.vector.tensor_copy(out=dst[:, sl], in_=pr[:])
                    else:
                        nc.scalar.copy(out=dst[:, sl], in_=pr[:])

            # s = psl1 * (1/count) rows; bias rows t/s appended below
            s1 = mm.tile([R + NP, C], F32R, tag="s", name="s1", bufs=2)
            nc.sync.dma_start(out=s1[R:R + NP, :], in_=tt9[0][:])  # reshape DMA
            for n in range(NCH):
                sl = slice(512 * n, 512 * (n + 1))
                nc.scalar.activation(out=s1[0:R, sl], in_=psl1[n][:], func=AF.Copy,
                                     scale=mrec72[:, 0:1])

            # bmm (with bias rows) + BN1 scale + relu, wave-ordered
            x1 = {}
            pos1 = {}
            for br, bd in (("m", BDre), ("s", BDse)):
                x1[br] = mm.tile([R, C], F32, tag=f"x1{br}", name=f"x1{br}")
                pos1[br] = []
                for n in range(NCH):
                    sl = slice(512 * n, 512 * (n + 1))
                    po = ps.tile([R, 512], F32, tag="ps", name=f"po1{br}{n}")
                    nc.tensor.matmul(po[:], bd[:], s1[:, sl], start=True, stop=True)
                    pos1[br].append(po)
            for br in ("m", "s"):
                for n in range(NCH):
                    sl = slice(512 * n, 512 * (n + 1))
                    nc.vector.tensor_tensor(x1[br][:, sl], pos1[br][n][:],
                                            srep1[:, sl], OP.mult)
            for br in ("m", "s"):
                for n in range(NCH):
                    sl = slice(512 * n, 512 * (n + 1))
                    nc.scalar.activation(out=x1[br][:, sl], in_=x1[br][:, sl],
                                         func=AF.Relu)

            # transpose x1 for layer 2
            x1T = {}
            for br in ("m", "s"):
                xt1 = mm.tile([128, KT, R], F32R,
                              tag=("mfT" if br == "m" else "x1Ts"), name=f"x1T{br}")
                for kt in range(KT):
                    pt1 = ps.tile([128, R], F32, tag="ps", name=f"pt1{br}{kt}")
                    nc.tensor.transpose(pt1[:], x1[br][:, 128 * kt:128 * (kt + 1)],
                                        ident[0:R, 0:R])
                    if kt % 2 == 0:
                        nc.vector.tensor_copy(out=xt1[:, kt, :], in_=pt1[:])
                    else:
                        nc.scalar.copy(out=xt1[:, kt, :], in_=pt1[:])
                x1T[br] = xt1

            # ---- gb/gn folds (needed only for outputs; emitted late) -------
            gbt, gnt = {}, {}
            for k in ("g", "b", "rm", "rv"):
                t = sp.tile([KT, 128], F32, tag=f"gb_{k}", name=f"gbl_{k}")
                nc.sync.dma_start(out=t[:],
                                  in_=gb[k].rearrange("(j lo) -> j lo", lo=128))
                gbt[k] = t
                t2 = sp.tile([KT, 128], F32, tag=f"gn_{k}", name=f"gnl_{k}")
                nc.sync.dma_start(out=t2[:],
                                  in_=gn[k].rearrange("(j lo) -> j lo", lo=128))
                gnt[k] = t2

            gfs = sp.tile([KT, 128], F32, tag="gf_s", name="gfs")
            gft = sp.tile([KT, 128], F32, tag="gf_t", name="gft")
            nc.vector.tensor_scalar_add(gfs[:], gbt["rv"][:], EPS)
            nc.scalar.activation(out=gfs[:], in_=gfs[:], func=AF.Sqrt)
            nc.vector.reciprocal(out=gfs[:], in_=gfs[:])
            nc.vector.tensor_mul(gfs[:], gfs[:], gbt["g"][:])
            nc.vector.tensor_mul(gft[:], gbt["rm"][:], gfs[:])
            nc.vector.tensor_sub(gft[:], gbt["b"][:], gft[:])
            nc.vector.tensor_scalar_mul(gfs[:], gfs[:], 1.0 / HW)  # GAP 1/HW
            for src, dst in ((gfs, sgb), (gft, tgb)):
                pt = ps.tile([128, KT], F32, tag="ps", name=f"pgb_{dst.name}")
                nc.tensor.transpose(pt[:], src[:], ident[0:KT, 0:KT])
                nc.vector.tensor_copy(out=dst[:], in_=pt[:])

            nfs = sp.tile([KT, 128], F32, tag="nf_s", name="nfs")
            nft = sp.tile([KT, 128], F32, tag="nf_t", name="nft")
            nc.vector.tensor_scalar_add(nfs[:], gnt["rv"][:], EPS)
            nc.scalar.activation(out=nfs[:], in_=nfs[:], func=AF.Sqrt)
            nc.vector.reciprocal(out=nfs[:], in_=nfs[:])
            nc.vector.tensor_mul(nfs[:], nfs[:], gnt["g"][:])
            nc.vector.tensor_mul(nft[:], gnt["rm"][:], nfs[:])
            nc.vector.tensor_sub(nft[:], gnt["b"][:], nft[:])
            nfsb = sp.tile([KT, 128], BF16, tag="nf_sb", name="nfsb")
            nftb = sp.tile([KT, 128], BF16, tag="nf_tb", name="nftb")
            nc.vector.tensor_copy(out=nfsb[:], in_=nfs[:])
            nc.vector.tensor_copy(out=nftb[:], in_=nft[:])
            for srcb, dst, nm in ((nfsb, sgn, "gn1s"), (nftb, tgn, "gn1t")):
                g1 = sp.tile([1, C], BF16, tag=nm, name=nm, bufs=1)
                nc.sync.dma_start(out=g1[:], in_=srcb[:])
                for n in range(NCH):
                    sl = slice(512 * n, 512 * (n + 1))
                    pg = ps.tile([BL, 512], F32, tag="ps",
                                 name=f"pgn_{dst.name}{n}")
                    nc.tensor.matmul(pg[:], ones_row8[:], g1[:, sl],
                                     start=True, stop=True)
                    nc.scalar.copy(out=dst[:, sl], in_=pg[:])

            # ========== GCN layer 2 (chases W2) + x_global interleaved =======
            # 1 x_global half per 2 W2 tiles early, the rest after: W2 lands
            # ~20us before the stream ends, so the layer-2 epilogue overlaps
            # the x_global tail and the kernel ends on the cheap GAP finish.
            def emit_xgt(i):
                b, hf = divmod(i, 2)
                xgt = stream.tile([128, JH, HW], F32R, tag="stream",
                                  name=f"xg{b}_{hf}", bufs=5)
                nc.gpsimd.dma_start(
                    out=xgt[:],
                    in_=xg_p[b, 128 * JH * hf:128 * JH * (hf + 1), :]
                    .rearrange("(j p) hw -> p j hw", p=128))
                j0 = JH * hf
                nc.vector.reduce_sum(out=G[:, b, j0:j0 + 6],
                                     in_=xgt[:, 0:6, :],
                                     axis=mybir.AxisListType.X)
                for j in range(6, JH):
                    nc.scalar.activation(out=dump[:], in_=xgt[:, j, :],
                                         func=AF.Copy,
                                         accum_out=G[:, b, j0 + j:j0 + j + 1])

            psl2 = {br: [ps.tile([R, 512], F32, tag="ps", name=f"psl2_{br}_{n}")
                         for n in range(NCH)] for br in ("m", "s")}
            xgi = 0
            for kt in range(KT):
                w = wp.tile([128, C], F32R, tag="w", name=f"w2_{kt}")
                nc.gpsimd.dma_start(out=w[:], in_=w2_p[128 * kt:128 * (kt + 1), :])
                for br in ("m", "s"):
                    for n in range(NCH):
                        nc.tensor.matmul(psl2[br][n][:], x1T[br][:, kt, :],
                                         w[:, 512 * n:512 * (n + 1)],
                                         start=(kt == 0), stop=(kt == KT - 1))
                if kt % 2 == 0 and xgi < 8:
                    emit_xgt(xgi)
                    xgi += 1

            # ---- layer-2 epilogue, wave-ordered and branch-interleaved ----
            s2 = {}
            for br in ("m", "s"):
                t = mm.tile([R + NP, C], F32R, tag="s", name=f"s2{br}", bufs=2)
                nc.sync.dma_start(out=t[R:R + NP, :], in_=tt9[1][:])
                for n in range(NCH):
                    sl = slice(512 * n, 512 * (n + 1))
                    if n % 2 == 0:
                        nc.vector.tensor_copy(out=t[0:R, sl], in_=psl2[br][n][:])
                    else:
                        nc.scalar.copy(out=t[0:R, sl], in_=psl2[br][n][:])
                s2[br] = t

            cat_off = {"m": 3 * C, "s": 3 * C + NP * C}
            bnf2 = sp.tile([BL, 2, C], F32, tag="bnf", name="bnf2", bufs=1)
            x2d, pos2 = {}, {}
            for br in ("m", "s"):
                x2d[br] = mm.tile([R, C], F32R, tag=f"x1{br}", name=f"x2{br}")
                pos2[br] = []
            for n in range(NCH):
                sl = slice(512 * n, 512 * (n + 1))
                for br in ("m", "s"):
                    po = ps.tile([R, 512], F32, tag="ps", name=f"po2{br}{n}")
                    nc.tensor.matmul(po[:], BDre[:], s2[br][:, sl],
                                     start=True, stop=True)
                    pos2[br].append(po)
            for n in range(NCH):
                sl = slice(512 * n, 512 * (n + 1))
                for br in ("m", "s"):
                    nc.vector.tensor_tensor(x2d[br][:, sl], pos2[br][n][:],
                                            srep2[:, sl], OP.mult)
            for br in ("m", "s"):
                for n in range(NCH):
                    sl = slice(512 * n, 512 * (n + 1))
                    nc.scalar.activation(out=x2d[br][:, sl], in_=x2d[br][:, sl],
                                         func=AF.Relu)
                off = cat_off[br]
                catv = out_p[:, off:off + NP * C].rearrange("b (p d) -> b p d", d=C)
                nc.sync.dma_start(out=catv[:, :, :], in_=x2d[br][:].bitcast(F32))
            pfs = {}
            for br in ("m", "s"):
                pfs[br] = []
                for n in range(NCH):
                    sl = slice(512 * n, 512 * (n + 1))
                    pf = ps.tile([BL, 512], F32, tag="ps", name=f"pf{br}{n}")
                    nc.tensor.matmul(pf[:], onesblk_r[:], x2d[br][:, sl],
                                     start=True, stop=True)
                    pfs[br].append(pf)

            # last x_global halves stream behind the epilogue's DVE waves
            for i in range(8, 2 * BL):
                emit_xgt(i)

            # ---- bnfeat_global out
            nc.vector.tensor_tensor(
                Gn[:], G[:], sgb[:, None, :].to_broadcast([128, BL, KT]), OP.mult)
            nc.vector.tensor_tensor(
                Gn[:], Gn[:], tgb[:, None, :].to_broadcast([128, BL, KT]), OP.add)
            pG = ps.tile([128, 128], F32, tag="ps", name="pG")
            nc.tensor.transpose(pG[:], Gn[:].rearrange("p b j -> p (b j)"), ident[:])
            nc.vector.tensor_copy(out=GT[:], in_=pG[:])
            nc.sync.dma_start(
                out=out_p[:, 0:C].rearrange("b (j p) -> b j p", p=128), in_=GT[:])

            # ---- bnfeat gcn/self means
            for br in ("m", "s"):
                bnf = bnf2[:, 0, :] if br == "m" else bnf2[:, 1, :]
                for n in range(NCH):
                    sl = slice(512 * n, 512 * (n + 1))
                    nc.vector.tensor_tensor(bnf[:, sl], pfs[br][n][:],
                                            sgn[:, sl], OP.mult)
                    nc.vector.tensor_tensor(bnf[:, sl], bnf[:, sl], tgn[:, sl],
                                            OP.add)
            nc.sync.dma_start(out=out_p[:, C:3 * C], in_=bnf2[:])

    legalize_waits(nc)
    return nc


_CACHE = {}


def kernel(_run_kwargs=None, **inputs):
    run_kwargs = _run_kwargs or {}
    if "nc" not in _CACHE:
        _CACHE["nc"] = build_bass()
    nc = _CACHE["nc"]

    B = inputs["x_global"].shape[0]
    n_cores = 8
    bl = B // n_cores

    rep_names = ["W1", "W2", "b1", "b2", "g1", "be1", "rm1", "rv1",
                 "g2", "be2", "rm2", "rv2",
                 "gb_g", "gb_b", "gb_rm", "gb_rv",
                 "gn_g", "gn_b", "gn_rm", "gn_rv"]

    in_maps = []
    for c in range(n_cores):
        sl = slice(c * bl, (c + 1) * bl)
        m = {
            "x_global": np.ascontiguousarray(
                inputs["x_global"][sl]).reshape(bl, C, HW).astype(np.float32),
            "x_gcn": np.ascontiguousarray(
                inputs["x_gcn"][sl]).reshape(bl, C, HW).astype(np.float32),
            "mask": np.ascontiguousarray(
                inputs["mask"][sl, 0]).astype(np.int32),
            "adj": np.ascontiguousarray(inputs["adj"][sl]).astype(np.float32),
        }
        for k in rep_names:
            m[k] = np.ascontiguousarray(inputs[k]).astype(np.float32)
        in_maps.append(m)

    from concourse.bass_utils import run_bass_kernel_spmd
    res = run_bass_kernel_spmd(nc, in_maps, list(range(n_cores)), **run_kwargs)
    out = np.concatenate([res.results[c]["out"] for c in range(n_cores)], axis=0)
    _CACHE["last_results"] = res
    return out
